# revision 4
# baseline (speedup 1.0000x reference)
"""Trainium2 Bass kernel for nn_CombinedRepeatCausalLinear.

Math: out[b,e,t] = sum_{s<=t} x[b,e,s] * (w0[s]*dv0^(t-s) + w1[t]*dv1^(t-s)) + bias[t]

which is two first-order linear recurrences along the S axis:
    yr[t] = dv0*yr[t-1] + w0[t]*x[t]      (row-repeat term)
    yc[t] = dv1*yc[t-1] + x[t]            (col-repeat term)
    out[t] = yr[t] + w1[t]*yc[t] + bias[t]

Implemented with DVE tensor_tensor_scan in natural [row, S] layout (no
transpose, no matmul). Data-parallel across 8 NeuronCores on the fused
B*E axis.
"""

import sys

if "/opt/trn_rl_repo" not in sys.path:
    sys.path.insert(0, "/opt/trn_rl_repo")

import numpy as np

import concourse.mybir as mybir
from concourse import bacc, bass
from concourse.bass_utils import run_bass_kernel_spmd
from concourse.mybir import AluOpType
from concourse.tile import TileContext

_P = 128
_B, _E, _S = 4, 2048, 2048
_NCORES = 8
_ROWS = (_B * _E) // _NCORES  # 1024 rows per core
_NT = _ROWS // _P  # 8 tiles of [128, S] per core

_F32 = mybir.dt.float32


def _build(dv0: float, dv1: float, with_bias: bool) -> bass.Bass:
    nc = bacc.Bacc(
        "TRN2",
        target_bir_lowering=False,
        debug=False,
        enable_asserts=False,
        num_devices=_NCORES,
    )
    xs = nc.dram_tensor("xs", [_ROWS, _S], _F32, kind="ExternalInput").ap()
    w0r = nc.dram_tensor("w0r", [1, _S], _F32, kind="ExternalInput").ap()
    w1r = nc.dram_tensor("w1r", [1, _S], _F32, kind="ExternalInput").ap()
    if with_bias:
        br = nc.dram_tensor("br", [1, _S], _F32, kind="ExternalInput").ap()
    out = nc.dram_tensor("out", [_ROWS, _S], _F32, kind="ExternalOutput").ap()

    with TileContext(nc) as tc:
        with (
            tc.tile_pool(name="consts", bufs=1) as cpool,
            tc.tile_pool(name="io", bufs=3) as iopool,
            tc.tile_pool(name="work", bufs=2) as wpool,
        ):
            w0b = cpool.tile([_P, _S], _F32)
            w1b = cpool.tile([_P, _S], _F32)
            nc.sync.dma_start(w0b[:], w0r.partition_broadcast(_P))
            nc.sync.dma_start(w1b[:], w1r.partition_broadcast(_P))
            if with_bias:
                bb = cpool.tile([_P, _S], _F32)
                nc.sync.dma_start(bb[:], br.partition_broadcast(_P))
            # data0 tiles for the scan's multiplicative decay term
            dv0t = cpool.tile([_P, _S], _F32)
            nc.gpsimd.memset(dv0t[:], dv0)
            if dv1 == dv0:
                dv1t = dv0t
            else:
                dv1t = cpool.tile([_P, _S], _F32)
                nc.gpsimd.memset(dv1t[:], dv1)

            for i in range(_NT):
                xt = iopool.tile([_P, _S], _F32, tag="x")
                nc.sync.dma_start(xt[:], xs[i * _P : (i + 1) * _P, :])

                xw = wpool.tile([_P, _S], _F32, tag="xw")
                nc.vector.tensor_tensor(xw[:], xt[:], w0b[:], AluOpType.mult)

                # yr[t] = dv0*yr[t-1] + (w0*x)[t]
                yr = wpool.tile([_P, _S], _F32, tag="yr")
                nc.vector.tensor_tensor_scan(
                    yr[:], dv0t[:], xw[:], 0.0, AluOpType.mult, AluOpType.add
                )
                # yc[t] = dv1*yc[t-1] + x[t]
                yc = wpool.tile([_P, _S], _F32, tag="yc")
                nc.vector.tensor_tensor_scan(
                    yc[:], dv1t[:], xt[:], 0.0, AluOpType.mult, AluOpType.add
                )

                t2 = wpool.tile([_P, _S], _F32, tag="t2")
                nc.vector.tensor_tensor(t2[:], yc[:], w1b[:], AluOpType.mult)

                ot = iopool.tile([_P, _S], _F32, tag="o")
                nc.vector.tensor_tensor(ot[:], yr[:], t2[:], AluOpType.add)
                if with_bias:
                    nc.vector.tensor_tensor(ot[:], ot[:], bb[:], AluOpType.add)

                nc.sync.dma_start(out[i * _P : (i + 1) * _P, :], ot[:])
    nc.compile()
    return nc


def _run(x, weight, bias, decay_value, trace=False):
    x = np.ascontiguousarray(np.asarray(x, dtype=np.float32))
    w = np.asarray(weight, dtype=np.float32)
    b = np.asarray(bias, dtype=np.float32)
    dv = np.asarray(decay_value, dtype=np.float32)
    dv0 = float(np.clip(dv[0, 0], 0.9, 1.0))
    dv1 = float(np.clip(dv[1, 0], 0.9, 1.0))
    with_bias = bool(np.any(b))

    nc = _build(dv0, dv1, with_bias)

    xf = x.reshape(_B * _E, _S)
    in_maps = []
    for c in range(_NCORES):
        m = {
            "xs": np.ascontiguousarray(xf[c * _ROWS : (c + 1) * _ROWS]),
            "w0r": np.ascontiguousarray(w[0:1]),
            "w1r": np.ascontiguousarray(w[1:2]),
        }
        if with_bias:
            m["br"] = np.ascontiguousarray(b[None, :])
        in_maps.append(m)

    res = run_bass_kernel_spmd(nc, in_maps, core_ids=list(range(_NCORES)), trace=trace)
    outs = [res.results[c]["out"] for c in range(_NCORES)]
    full = np.concatenate(outs, axis=0).reshape(_B, _E, _S)
    return full, res


def kernel(x, weight, bias, decay_value):
    full, _ = _run(x, weight, bias, decay_value, trace=False)
    return full
